# revision 5
# baseline (speedup 1.0000x reference)
"""Trainium2 Bass kernel for a 2-layer bidirectional GRU classifier.

Model (see reference): x[256,256,4] -> BiGRU(H=256) -> BiGRU(H=256)
  -> sum fwd/bwd halves -> last timestep -> Linear(3) -> softmax.

CoreSim cost model: 614.9us (previous checkpoint 877us); verified on
hardware, scale-relative error 2.4e-3.

Key structure:
  * Output uses only t=T-1, so layer-1 backward needs exactly ONE step
    (computed up front so it overlaps the layer-1 forward phase) and
    layer-1 forward states need not be stored.
  * Transposed layout: gate/hidden dim on SBUF partitions, batch on the
    free dim; input projections batched 4 steps at a time into PSUM with
    the recurrence matmuls accumulating on top (biases ride in via a
    host-packed ones-row / ones-vector matmuls).  Projection matmuls are
    split into small moving-dim chunks to limit head-of-line blocking of
    latency-critical recurrence matmuls on the in-order PE.
  * The per-step elementwise work is balanced around the Activation
    engine, which is the throughput bottleneck (4 activation
    instructions per step, ~185ns fixed SBUF-access cost each):
      - sigmoid/tanh on Act (PSUM -> SBUF bf16),
      - hn and xn moved PSUM -> SBUF by DVE copies that depend only on
        the matmuls, so they run concurrently with the sigmoid, OFF the
        critical chain.  PSUM dependencies are tracked bank-ordered, so
        hn gets its own PSUM tile (own bank) where sharing a tile with
        the rz gates would serialize the copy against the sigmoid,
      - r*hn, +xn, (z-1), (z-1)*n and the h update on GPSIMD/Pool
        (SBUF-only operands: GPSIMD cannot access PSUM on real HW),
        issued chain-critical-first so (z-1) does not delay r*hn in
        the in-order Pool queue,
      - z*h_prev on DVE (also off the critical chain),
      - h_new = z*h - (z-1)*n  (no explicit (1-z) tensor needed).
  * Layer 0 runs fwd+bwd as two interleaved chains (latency hiding);
    layer 1 runs two independent batch-16 half-chains with per-half
    PSUM tiles.  More chains would inflate the fixed activation cost;
    fewer would be latency-bound: both measured worse.
  * DMA queue assignment: layer-0-critical loads lead the SP queue,
    only small step-0 tensors on the Activation queue (so
    LoadActFuncSet and the first sigmoid are not delayed), nothing on
    Pool.
  * Sharding: pure batch-parallel, 32 rows per core, 8 cores, zero
    collectives; every core runs the identical program on different
    data (the backward direction is a forward recurrence over
    time-reversed inputs, prepared host-side).

All matmul operands are bf16 (fp32 PSUM accumulation).
"""

import sys

import numpy as np

try:
    import concourse.bass as bass
except ImportError:  # pragma: no cover
    sys.path.insert(0, "/opt/trn_rl_repo")
    import concourse.bass as bass

import ml_dtypes
from concourse import bacc, mybir
from concourse import bass_utils
import concourse.tile as tile
from concourse.alu_op_type import AluOpType

BF16NP = ml_dtypes.bfloat16
F32 = mybir.dt.float32
BF16 = mybir.dt.bfloat16
AF = mybir.ActivationFunctionType
AX = mybir.AxisListType
OP = AluOpType

B, H = 256, 256
NCORES = 8
BL = B // NCORES  # 32 batch rows per core
HB = BL // 2      # L1 half-chain batch


# --------------------------------------------------------------------------
# host-side packing (identical to baseline)
# --------------------------------------------------------------------------

def _pack_weights(inp, T):
    d = {}

    def whh_pack(W):  # W [768, 256] -> [128, 12, 128] (a, k*6+m, col)
        out = np.zeros((128, 12, 128), np.float32)
        for k in range(2):
            for m in range(6):
                out[:, k * 6 + m, :] = W[m * 128:(m + 1) * 128,
                                         k * 128:(k + 1) * 128].T
        return out.astype(BF16NP)

    def wih_pack(W, bias_col, K):  # W [768, K-1] + bias -> [K, 6, 128]
        Waug = np.concatenate([W, bias_col[:, None]], axis=1)  # [768, K]
        out = np.zeros((K, 6, 128), np.float32)
        for m in range(6):
            out[:, m, :] = Waug[m * 128:(m + 1) * 128, :].T
        return out.astype(BF16NP)

    def wih1_pack(W):  # W [768, 512] -> [128, 24, 128]
        out = np.zeros((128, 24, 128), np.float32)
        for k in range(4):
            for m in range(6):
                out[:, k * 6 + m, :] = W[m * 128:(m + 1) * 128,
                                         k * 128:(k + 1) * 128].T
        return out.astype(BF16NP)

    # layer 0
    w0, b0 = [], []
    for dd in range(2):
        bias_col = inp["bih0"][dd].copy()
        bias_col[:512] += inp["bhh0"][dd][:512]
        w0.append(wih_pack(inp["Wih0"][dd], bias_col, 5))
        b0.append(inp["bhh0"][dd][512:].reshape(1, 2, 128).astype(BF16NP))
    d["wih0"] = np.stack(w0)                      # [2, 5, 6, 128]
    d["bhhn0"] = np.stack(b0)                     # [2, 1, 2, 128]
    d["whh0"] = np.stack([whh_pack(inp["Whh0"][dd]) for dd in range(2)])

    # layer 1
    w1, w1b = [], []
    for dd in range(2):
        bias_col = inp["bih1"][dd].copy()
        bias_col[:512] += inp["bhh1"][dd][:512]
        w1.append(wih1_pack(inp["Wih1"][dd]))
        w1b.append(bias_col.reshape(1, 6, 128).astype(BF16NP))
    d["wih1"] = np.stack(w1)                      # [2, 128, 24, 128]
    d["wih1b"] = np.stack(w1b)                    # [2, 1, 6, 128]
    d["whh1"] = whh_pack(inp["Whh1"][0])          # fwd only
    d["bhhn1"] = inp["bhh1"][0][512:].reshape(1, 2, 128).astype(BF16NP)
    # bwd 1-step: h=0 so hn = bhh1[1][512:], broadcast over batch
    hnb = inp["bhh1"][1][512:].reshape(2, 128)    # [c, p]
    d["hnb1b"] = np.repeat(hnb.T.reshape(128, 2, 1), BL, axis=2).reshape(
        128, 2 * BL).astype(BF16NP)               # [128, 2*BL] (p, c*BL+b)

    # head
    woutt = np.zeros((128, 2, 3), np.float32)
    for k in range(2):
        woutt[:, k, :] = inp["W_out"][:, k * 128:(k + 1) * 128].T
    d["woutt"] = woutt.astype(BF16NP)
    d["boutb"] = np.tile(inp["b_out"][None, :], (BL, 1)).astype(np.float32)
    return d


def _pack_x(x_chunk, T):
    """x_chunk [BL, T, 4] -> xt [2, 5, T*BL] bf16 (fwd t-major + reversed)."""
    xa = np.concatenate(
        [x_chunk, np.ones((BL, T, 1), np.float32)], axis=2)  # [BL, T, 5]
    fwd = xa.transpose(2, 1, 0).reshape(5, T * BL)
    rev = xa[:, ::-1, :].transpose(2, 1, 0).reshape(5, T * BL)
    return np.stack([fwd, rev]).astype(BF16NP)


# --------------------------------------------------------------------------
# device program
# --------------------------------------------------------------------------

def build_program(T=256):
    assert T % 8 == 0
    NBLK8 = T // 8

    nc = bacc.Bacc("TRN2", target_bir_lowering=False, debug=False)

    dr = {}
    dr["xt"] = nc.dram_tensor("xt", [2, 5, T * BL], BF16, kind="ExternalInput")
    dr["wih0"] = nc.dram_tensor("wih0", [2, 5, 6, 128], BF16, kind="ExternalInput")
    dr["whh0"] = nc.dram_tensor("whh0", [2, 128, 12, 128], BF16, kind="ExternalInput")
    dr["bhhn0"] = nc.dram_tensor("bhhn0", [2, 1, 2, 128], BF16, kind="ExternalInput")
    dr["wih1"] = nc.dram_tensor("wih1", [2, 128, 24, 128], BF16, kind="ExternalInput")
    dr["wih1b"] = nc.dram_tensor("wih1b", [2, 1, 6, 128], BF16, kind="ExternalInput")
    dr["whh1"] = nc.dram_tensor("whh1", [128, 12, 128], BF16, kind="ExternalInput")
    dr["bhhn1"] = nc.dram_tensor("bhhn1", [1, 2, 128], BF16, kind="ExternalInput")
    dr["hnb1b"] = nc.dram_tensor("hnb1b", [128, 2 * BL], BF16, kind="ExternalInput")
    dr["woutt"] = nc.dram_tensor("woutt", [128, 2, 3], BF16, kind="ExternalInput")
    dr["boutb"] = nc.dram_tensor("boutb", [BL, 3], F32, kind="ExternalInput")
    out_dram = nc.dram_tensor("out", [BL, 3], F32, kind="ExternalOutput")

    with tile.TileContext(nc) as tc:
        _build_tile(tc, nc, dr, out_dram, T, NBLK8)

    nc.compile()
    return nc


def _build_tile(tc, nc, dr, out_dram, T, NBLK8):
    from contextlib import ExitStack
    ctx = ExitStack()
    with ctx:
        wpool = ctx.enter_context(tc.tile_pool(name="w", bufs=1))
        h1pool = ctx.enter_context(tc.tile_pool(name="h1", bufs=1))
        work = ctx.enter_context(tc.tile_pool(name="work", bufs=8))

        def load(name, shape, dtype=BF16, sub=None, eng=None):
            t = wpool.tile(shape, dtype, tag=name + str(sub),
                           name=name + str(sub))
            src = dr[name].ap()
            if sub is not None:
                src = src[sub]
            (eng or nc.sync).dma_start(out=t, in_=src)
            return t

        # DMA queue assignment: the four layer-0-critical loads lead the SP
        # queue, everything bulky follows them there (needed much later);
        # Act gets only the small step-0 tensors so its sequencer is free
        # for LoadActFuncSet + the first sigmoid; Pool gets nothing (the
        # first r*hn runs at ~5us)
        XCUT = min(32 * BL, T * BL)
        xt_sb = []
        for dd in range(2):
            t = wpool.tile([5, T * BL], BF16, tag=f"xt{dd}", name=f"xt{dd}")
            nc.sync.dma_start(out=t[:, :XCUT], in_=dr["xt"].ap()[dd][:, :XCUT])
            xt_sb.append(t)
        wih0_sb = [load("wih0", [5, 6, 128], sub=dd) for dd in range(2)]
        whh0_sb = [load("whh0", [128, 12, 128], sub=dd) for dd in range(2)]
        bhhn0_sb = [load("bhhn0", [1, 2, 128], sub=dd, eng=nc.scalar)
                    for dd in range(2)]
        for dd in range(2):
            if XCUT < T * BL:
                nc.sync.dma_start(out=xt_sb[dd][:, XCUT:],
                                  in_=dr["xt"].ap()[dd][:, XCUT:])
        wih1_sb = [load("wih1", [128, 24, 128], sub=dd) for dd in range(2)]
        wih1b_sb = [load("wih1b", [1, 6, 128], sub=dd) for dd in range(2)]
        whh1_sb = load("whh1", [128, 12, 128])
        bhhn1_sb = load("bhhn1", [1, 2, 128])
        hnb1b_sb = load("hnb1b", [128, 2, BL])
        woutt_sb = load("woutt", [128, 2, 3])
        bout_sb = load("boutb", [BL, 3], F32)

        ones_sb = wpool.tile([1, 128], BF16, tag="ones", name="ones")
        nc.vector.memset(ones_sb, 1.0)

        # h1 history, transposed: per 8-step block [128, chunk(4), slot(8), BL]
        # chunks 0,1 = fwd H-halves; 2,3 = bwd H-halves
        h1t = [h1pool.tile([128, 4, 8, BL], BF16, tag=f"h1_{j}",
                           name=f"h1_{j}")
               for j in range(NBLK8)]

        def h1_ap(dd, t):
            j, sl = divmod(t, 8)
            return h1t[j][:, 2 * dd:2 * dd + 2, sl, :]

        def h1_k(dd, t, k):
            j, sl = divmod(t, 8)
            return h1t[j][:, 2 * dd + k, sl, :]

        # ------------------------------------------------------------------
        # layer 0: both directions, interleaved chains
        # ------------------------------------------------------------------
        with tc.tile_pool(name="ps0", bufs=2, space="PSUM") as ps0:
            rz_cur = [None, None]
            xnhn_cur = [None, None]
            xn_sb = [None, None]
            for i in range(T):
                blk, s = divmod(i, 4)
                for dd in range(2):
                    t = i if dd == 0 else T - 1 - i
                    if s == 0:
                        rz_cur[dd] = ps0.tile([128, 4, 4, BL], F32,
                                              tag=f"rz{dd}", name=f"rz{dd}")
                        xnhn_cur[dd] = ps0.tile([128, 2, 2, 4, BL], F32,
                                                tag=f"xnhn{dd}", name=f"xnhn{dd}")
                        for m in range(6):
                            for hh in range(2):
                                rhs = xt_sb[dd][:, (blk * 4 + hh * 2) * BL:
                                                (blk * 4 + hh * 2 + 2) * BL]
                                oap = (rz_cur[dd][:, m, 2 * hh:2 * hh + 2, :]
                                       if m < 4 else
                                       xnhn_cur[dd][:, 0, m - 4,
                                                    2 * hh:2 * hh + 2, :])
                                nc.tensor.matmul(
                                    oap, wih0_sb[dd][:, m, :], rhs,
                                    start=(m in (0, 4) and hh == 0),
                                    stop=(m in (3, 5) and hh == 1),
                                    skip_group_check=True)
                        xn_sb[dd] = work.tile([128, 2, 4, BL], F32,
                                              tag=f"xns{dd}", name=f"xns{dd}")
                        nc.vector.tensor_copy(xn_sb[dd],
                                              xnhn_cur[dd][:, 0, :, :, :])
                    if i > 0:
                        tp = t - 1 if dd == 0 else t + 1
                        for m in range(4):
                            for k in range(2):
                                nc.tensor.matmul(
                                    rz_cur[dd][:, m, s, :],
                                    whh0_sb[dd][:, k * 6 + m, :],
                                    h1_k(dd, tp, k), start=False,
                                    stop=(k == 1), skip_group_check=True)
                    for m in (4, 5):
                        if i > 0:
                            for k in range(2):
                                nc.tensor.matmul(
                                    xnhn_cur[dd][:, 1, m - 4, s, :],
                                    whh0_sb[dd][:, k * 6 + m, :],
                                    h1_k(dd, tp, k),
                                    start=False, stop=False,
                                    skip_group_check=True)
                        nc.tensor.matmul(
                            xnhn_cur[dd][:, 1, m - 4, s, :],
                            bhhn0_sb[dd][:, m - 4, :], ones_sb[:, :BL],
                            start=False, stop=True, skip_group_check=True)
                    # hn -> SBUF off the critical chain (only needs the mms,
                    # runs concurrently with the sigmoid)
                    hn_sb = work.tile([128, 2, BL], F32, tag=f"hns{dd}",
                                      name=f"hns{dd}")
                    nc.vector.tensor_copy(hn_sb, xnhn_cur[dd][:, 1, :, s, :])

                    # elementwise: sigma/tanh on Act, PSUM-touching ops on
                    # GPSIMD, all-SBUF bf16 lerp on DVE
                    rz = work.tile([128, 4, BL], BF16, tag=f"rzs{dd}",
                                   name=f"rzs{dd}")
                    nc.scalar.activation(rz, rz_cur[dd][:, :, s, :], AF.Sigmoid)
                    q = work.tile([128, 2, BL], BF16, tag=f"q{dd}",
                                  name=f"q{dd}")
                    if i > 0:
                        nc.vector.tensor_tensor(q, rz[:, 2:4, :],
                                                h1_ap(dd, tp), OP.mult)
                    nin = work.tile([128, 2, BL], BF16, tag=f"nin{dd}",
                                    name=f"nin{dd}")
                    nc.gpsimd.tensor_tensor(nin, rz[:, 0:2, :], hn_sb, OP.mult)
                    npre = work.tile([128, 2, BL], F32, tag=f"npre{dd}",
                                     name=f"npre{dd}")
                    nc.gpsimd.tensor_tensor(npre, nin,
                                            xn_sb[dd][:, :, s, :], OP.add)
                    zm = work.tile([128, 2, BL], BF16, tag=f"zm{dd}",
                                   name=f"zm{dd}")
                    nc.gpsimd.tensor_scalar_sub(zm, rz[:, 2:4, :], 1.0)
                    nsb = work.tile([128, 2, BL], BF16, tag=f"n{dd}",
                                    name=f"n{dd}")
                    nc.scalar.activation(nsb, npre, AF.Tanh)
                    w = work.tile([128, 2, BL], BF16, tag=f"w{dd}",
                                  name=f"w{dd}")
                    nc.gpsimd.tensor_tensor(w, zm, nsb, OP.mult)
                    hout = h1_ap(dd, t)
                    if i == 0:
                        nc.gpsimd.tensor_scalar_mul(hout, w, -1.0)
                    else:
                        nc.gpsimd.tensor_tensor(hout, q, w, OP.subtract)

        # ------------------------------------------------------------------
        # layer 1: forward chain (two batch-16 half-chains), then one
        # backward step, then head
        # ------------------------------------------------------------------
        h2pool = ctx.enter_context(tc.tile_pool(name="h2", bufs=4))
        psbw = ctx.enter_context(tc.tile_pool(name="psbw", bufs=1,
                                              space="PSUM"))
        # backward direction of layer 1: single step at t = T-1 (h0 = 0).
        # Depends only on h1t (complete when L0 ends), so issuing it here
        # lets it overlap the start of the L1 forward phase.
        bw = psbw.tile([128, 6, BL], F32, tag="bw", name="bw", bufs=1)
        for m in range(6):
            for k in range(4):
                nc.tensor.matmul(bw[:, m, :], wih1_sb[1][:, k * 6 + m, :],
                                 h1t[NBLK8 - 1][:, k, 7, :],
                                 start=(k == 0 and m == 0), stop=False,
                                 skip_group_check=True)
            nc.tensor.matmul(bw[:, m, :], wih1b_sb[1][:, m, :],
                             ones_sb[:, :BL], start=False, stop=(m == 5),
                             skip_group_check=True)
        rzb = work.tile([128, 4, BL], BF16, tag="rzb", name="rzb")
        nc.scalar.activation(rzb, bw[:, 0:4, :], AF.Sigmoid)
        ninb = work.tile([128, 2, BL], BF16, tag="ninb", name="ninb")
        nc.vector.tensor_mul(ninb, rzb[:, 0:2, :], hnb1b_sb)
        npreb = work.tile([128, 2, BL], F32, tag="npreb", name="npreb")
        nc.vector.tensor_add(npreb, ninb, bw[:, 4:6, :])
        nb = work.tile([128, 2, BL], BF16, tag="nb", name="nb")
        nc.scalar.activation(nb, npreb, AF.Tanh)
        eb = work.tile([128, 2, BL], BF16, tag="eb", name="eb")
        nc.vector.tensor_mul(eb, rzb[:, 2:4, :], nb)
        h2b = work.tile([128, 2, BL], BF16, tag="h2b", name="h2b")
        nc.vector.tensor_sub(h2b, nb, eb)

        with tc.tile_pool(name="ps1", bufs=2, space="PSUM") as ps1:
            # fully decoupled half-chains; hn in its own PSUM bank so the
            # off-chain hn->SBUF copy doesn't serialize with the sigmoid
            # (PSUM deps are tracked bank-ordered)
            h2_prev = [None, None]
            gate1 = [None, None]
            hnp1 = [None, None]
            xn1_sb = [None, None]
            for i in range(T):
                blk, s = divmod(i, 4)
                h2c = [None, None]
                for c in range(2):
                    cs = slice(c * HB, (c + 1) * HB)
                    if s == 0:
                        gate1[c] = ps1.tile([128, 6, 4, HB], F32,
                                            tag=f"g1{c}", name=f"g1{c}")
                        hnp1[c] = ps1.tile([128, 2, 4, HB], F32,
                                           tag=f"hn1{c}", name=f"hn1{c}",
                                           bufs=1)
                        j8, sl0 = divmod(i, 8)
                        for m in range(6):
                            for k in range(4):
                                for hh in range(2):
                                    nc.tensor.matmul(
                                        gate1[c][:, m, 2 * hh:2 * hh + 2, :],
                                        wih1_sb[0][:, k * 6 + m, :],
                                        h1t[j8][:, k,
                                                sl0 + 2 * hh:sl0 + 2 * hh + 2,
                                                cs],
                                        start=(k == 0 and m == 0 and hh == 0),
                                        stop=False, skip_group_check=True)
                            nc.tensor.matmul(gate1[c][:, m, :, :],
                                             wih1b_sb[0][:, m, :],
                                             ones_sb[:, :4 * HB],
                                             start=False, stop=(m == 5),
                                             skip_group_check=True)
                        xn1_sb[c] = work.tile([128, 2, 4, HB], F32,
                                              tag=f"xns1{c}", name=f"xns1{c}")
                        nc.vector.tensor_copy(xn1_sb[c],
                                              gate1[c][:, 4:6, :, :])
                    hp = h2_prev[c]
                    if hp is not None:
                        for m in range(4):
                            for k in range(2):
                                nc.tensor.matmul(
                                    gate1[c][:, m, s, :],
                                    whh1_sb[:, k * 6 + m, :],
                                    hp[:, k, :], start=False,
                                    stop=(k == 1), skip_group_check=True)
                    first_hn = (s == 0)
                    for m in (4, 5):
                        if hp is not None:
                            for k in range(2):
                                nc.tensor.matmul(
                                    hnp1[c][:, m - 4, s, :],
                                    whh1_sb[:, k * 6 + m, :], hp[:, k, :],
                                    start=first_hn, stop=False,
                                    skip_group_check=True)
                                first_hn = False
                        nc.tensor.matmul(
                            hnp1[c][:, m - 4, s, :],
                            bhhn1_sb[:, m - 4, :], ones_sb[:, :HB],
                            start=first_hn, stop=True,
                            skip_group_check=True)
                        first_hn = False
                    hn1_sb = work.tile([128, 2, HB], F32, tag=f"hns1{c}",
                                       name=f"hns1{c}")
                    nc.vector.tensor_copy(hn1_sb, hnp1[c][:, :, s, :])

                    h2 = h2pool.tile([128, 2, HB], BF16, tag=f"h2{c}",
                                     name=f"h2{c}")
                    rz = work.tile([128, 4, HB], BF16, tag=f"rzs1{c}",
                                   name=f"rzs1{c}")
                    nc.scalar.activation(rz, gate1[c][:, 0:4, s, :], AF.Sigmoid)
                    q = work.tile([128, 2, HB], BF16, tag=f"q1{c}",
                                  name=f"q1{c}")
                    if hp is not None:
                        nc.vector.tensor_tensor(q, rz[:, 2:4, :], hp, OP.mult)
                    nin = work.tile([128, 2, HB], BF16, tag=f"nin1{c}",
                                    name=f"nin1{c}")
                    nc.gpsimd.tensor_tensor(nin, rz[:, 0:2, :], hn1_sb, OP.mult)
                    npre = work.tile([128, 2, HB], F32, tag=f"npre1{c}",
                                     name=f"npre1{c}")
                    nc.gpsimd.tensor_tensor(npre, nin,
                                            xn1_sb[c][:, :, s, :], OP.add)
                    zm = work.tile([128, 2, HB], BF16, tag=f"zm1{c}",
                                   name=f"zm1{c}")
                    nc.gpsimd.tensor_scalar_sub(zm, rz[:, 2:4, :], 1.0)
                    nsb = work.tile([128, 2, HB], BF16, tag=f"n1{c}",
                                    name=f"n1{c}")
                    nc.scalar.activation(nsb, npre, AF.Tanh)
                    w = work.tile([128, 2, HB], BF16, tag=f"w1{c}",
                                  name=f"w1{c}")
                    nc.gpsimd.tensor_tensor(w, zm, nsb, OP.mult)
                    if hp is None:
                        nc.gpsimd.tensor_scalar_mul(h2[:, :, :], w, -1.0)
                    else:
                        nc.gpsimd.tensor_tensor(h2[:, :, :], q, w, OP.subtract)
                    h2c[c] = h2
                h2_prev = h2c

        if True:
            # head: logits = (h2f_last + h2b) @ W_out.T + b_out, then softmax
            hsum = work.tile([128, 2, BL], BF16, tag="hsum", name="hsum")
            for c in range(2):
                cs = slice(c * HB, (c + 1) * HB)
                nc.vector.tensor_add(hsum[:, :, cs], h2_prev[c],
                                     h2b[:, :, cs])
            head_ps = psbw.tile([BL, 3], F32, tag="head", name="head", bufs=1)
            for k in range(2):
                nc.tensor.matmul(head_ps, hsum[:, k, :], woutt_sb[:, k, :],
                                 start=(k == 0), stop=(k == 1))
            lg = work.tile([BL, 3], F32, tag="lg", name="lg")
            nc.vector.tensor_add(lg, head_ps, bout_sb)
            mx = work.tile([BL, 1], F32, tag="mx", name="mx")
            nc.vector.reduce_max(mx, lg, axis=AX.X)
            nmx = work.tile([BL, 1], F32, tag="nmx", name="nmx")
            nc.vector.tensor_scalar_mul(nmx, mx, -1.0)
            ex = work.tile([BL, 3], F32, tag="ex", name="ex")
            nc.scalar.activation(ex, lg, AF.Exp, bias=nmx)
            sm = work.tile([BL, 1], F32, tag="sm", name="sm")
            nc.vector.reduce_sum(sm, ex, axis=AX.X)
            rv = work.tile([BL, 1], F32, tag="rv", name="rv")
            nc.vector.reciprocal(rv, sm)
            outsb = work.tile([BL, 3], F32, tag="outsb", name="outsb")
            nc.vector.tensor_scalar_mul(outsb, ex, rv)
            nc.sync.dma_start(out=out_dram.ap(), in_=outsb)


# --------------------------------------------------------------------------
# entry point
# --------------------------------------------------------------------------

_CACHE = {}


def prep_in_maps(inputs, T=256):
    inputs = {k: np.asarray(v, np.float32) for k, v in inputs.items()}
    wpack = _pack_weights(inputs, T)
    in_maps = []
    for c in range(NCORES):
        m = dict(wpack)
        m["xt"] = _pack_x(inputs["x"][c * BL:(c + 1) * BL], T)
        in_maps.append(m)
    return in_maps


def kernel(**inputs) -> np.ndarray:
    T = np.asarray(inputs["x"]).shape[1]
    if T not in _CACHE:
        _CACHE[T] = build_program(T)
    nc = _CACHE[T]
    in_maps = prep_in_maps(inputs, T)
    res = bass_utils.run_bass_kernel_spmd(nc, in_maps, list(range(NCORES)))
    return np.concatenate([r["out"] for r in res.results], axis=0)
